# revision 4
# baseline (speedup 1.0000x reference)
"""Causal multi-head attention with relative position bias on 8 Trainium2
NeuronCores.

Problem (full shapes): x[2,2048,1024], rel_bias[16,2048,2048],
w_qkv[1024,3072], b_qkv[3072], w_out[1024,1024], b_out[1024].

Sharding: core = (batch, head-group): 2 batches x 4 head-groups of 4 heads.
Each core computes q/k/v projections for its 4 heads, causal attention with
rel-bias, and a partial output projection through its heads' rows of w_out.
Host sums the 4 partial outputs per batch (the tensor-parallel reduce) and
adds b_out.

Device kernel design notes:
- Scores are computed TRANSPOSED (scoresT[kj,qi] = k.q) so no on-chip
  transposes are needed anywhere: softmax reduction over keys becomes a
  matmul contraction, handled by appending a ones-column to V; the PV matmul
  directly produces the transposed attention output that the out-projection
  needs as its stationary operand.
- exp(score + bias) = exp(score) * exp(bias): host precomputes exp(rel_biasT)
  in bf16 with the causal mask baked in as exact zeros. ACT does a pure exp
  straight from PSUM; DVE multiplies two bf16 SBUF operands at 2x rate.
- All matmul operands are bf16 (PSUM accumulation is fp32); softmax
  denominators, reciprocals and the normalization are fp32.
- The per-query normalization 1/denom is broadcast across partitions with a
  stride-0 SBUF->SBUF DMA and applied to the small attention output, not the
  big probability matrix.
"""

import math
import sys
import types
from contextlib import ExitStack

import ml_dtypes
import numpy as np

B, S, D = 2, 2048, 1024
NH, HD = 16, 64
NCORES = 8
HPC = 4  # heads per core (2 pairs)

_BF16 = ml_dtypes.bfloat16


def _install_ntff_hook():
    """concourse.bass_utils imports antenv.axon_hooks for NTFF tracing under
    axon; this container's antenv lacks that module. Provide it, backed by
    the ctypes hook from trn_agent_boot (if present)."""
    if "antenv.axon_hooks" in sys.modules:
        return
    try:
        import antenv
    except ImportError:
        return
    mod = types.ModuleType("antenv.axon_hooks")
    mod._hook = None
    mod.set_axon_ntff_profile_hook = lambda h: setattr(mod, "_hook", h)
    mod.get_axon_ntff_profile_hook = lambda: mod._hook
    sys.modules["antenv.axon_hooks"] = mod
    antenv.axon_hooks = mod
    try:
        from trn_agent_boot.trn_boot import _ntff_profile_via_ctypes

        h = _ntff_profile_via_ctypes("/opt/axon/libaxon_pjrt.so")
        if h is not None:
            mod._hook = h
    except Exception:
        pass


KC = D // 128   # 8 contraction chunks for the projections
NS4 = S // 512  # 4 s-superblocks
NSC = S // 128  # 16 s-chunks


def _phase_load(ctx, tc, nc, d, has_bqk, has_bv, st):
    """DMA weights + xT into persistent SBUF tiles."""
    from concourse import mybir
    bf = mybir.dt.bfloat16

    xt_pool = ctx.enter_context(tc.tile_pool(name="xt", bufs=KC))
    wqk_pool = ctx.enter_context(tc.tile_pool(name="wqk", bufs=KC))
    wv_pool = ctx.enter_context(tc.tile_pool(name="wv", bufs=KC))
    wo_pool = ctx.enter_context(tc.tile_pool(name="wo", bufs=2))
    const_pool = ctx.enter_context(tc.tile_pool(name="consts", bufs=1))

    st.ones_row = const_pool.tile([1, 512], bf)
    nc.gpsimd.memset(st.ones_row[:], 1.0)
    # selection rows for the denominator broadcast: selA hits partitions
    # 0-63, selB partitions 64-127 (K=1 matmuls accumulate both)
    sel_f32 = const_pool.tile([1, 256], mybir.dt.float32)
    nc.gpsimd.memset(sel_f32[:], 0.0)
    nc.gpsimd.memset(sel_f32[0:1, 0:64], 1.0)
    nc.gpsimd.memset(sel_f32[0:1, 192:256], 1.0)
    st.sel_f32r = const_pool.tile([1, 256], mybir.dt.float32r)
    nc.vector.tensor_copy(st.sel_f32r[:], sel_f32[:])

    st.wqk_t, st.xt_t, st.wv_t = [], [], []
    for k in range(KC):
        w = wqk_pool.tile([128, 512], bf)
        nc.sync.dma_start(w[:], d.wqk[k * 128:(k + 1) * 128, :])
        st.wqk_t.append(w)
        xt = xt_pool.tile([128, S], bf)
        nc.sync.dma_start(xt[:], d.xT[k * 128:(k + 1) * 128, :])
        st.xt_t.append(xt)
    for k in range(KC):
        # wv is first consumed ~30us in; keep it out of the critical
        # DMA prefix that the first qk accumulation chain waits on
        wv = wv_pool.tile([128, 260], bf)
        nc.sync.dma_start(wv[:], d.wv[k * 128:(k + 1) * 128, :])
        st.wv_t.append(wv)
    st.wo_t = []
    for p in range(2):
        w = wo_pool.tile([128, D], bf)
        nc.sync.dma_start(w[:], d.wo[p])
        st.wo_t.append(w)
    if has_bqk:
        st.bqk_sb = []
        for m in range(4):
            t = const_pool.tile([1, 128], bf, name=f"bqk{m}", tag=f"bqk{m}")
            nc.sync.dma_start(t[:], d.bqk[m:m + 1, :])
            st.bqk_sb.append(t)
    if has_bv:
        st.bv_sb = const_pool.tile([1, 260], bf)
        nc.sync.dma_start(st.bv_sb[:], d.bv[:])


def _phase_proj(ctx, tc, nc, has_bqk, has_bv, st):
    """qkv projections.

    qkT[m][r, s]: m-chunks 0..3 = [q pair0 | k pair0 | q pair1 | k pair1];
    within a chunk rows 0-63 = first head of the pair, 64-127 = second.
    v_t[si]: [128, 260] bf16, 4 slots of 65 cols (64 v-cols + ones col).
    """
    from concourse import mybir
    bf = mybir.dt.bfloat16
    f32 = mybir.dt.float32

    qkT_pool = ctx.enter_context(tc.tile_pool(name="qkT", bufs=4))
    v_pool = ctx.enter_context(tc.tile_pool(name="vsb", bufs=NSC))
    st.qkT_t = [qkT_pool.tile([128, S], bf, name="qkT", tag="qkT") for _ in range(4)]
    st.v_t = [v_pool.tile([128, 260], bf, name="vsb", tag="vsb") for _ in range(NSC)]

    def emit_qk(qk_ps, m):
        for s4 in range(NS4):
            ps = qk_ps.tile([128, 512], f32, name="qkps", tag="qkps")
            for k in range(KC):
                nc.tensor.matmul(
                    ps[:],
                    st.wqk_t[k][:, m * 128:(m + 1) * 128],
                    st.xt_t[k][:, s4 * 512:(s4 + 1) * 512],
                    start=(k == 0),
                    stop=(k == KC - 1 and not has_bqk),
                )
            if has_bqk:
                nc.tensor.matmul(
                    ps[:], st.bqk_sb[m][:], st.ones_row[:, :],
                    start=False, stop=True,
                )
            nc.vector.tensor_copy(
                st.qkT_t[m][:, s4 * 512:(s4 + 1) * 512], ps[:])

    with tc.tile_pool(name="qk_ps", bufs=4, space="PSUM") as qk_ps, \
         tc.tile_pool(name="v_ps", bufs=3, space="PSUM") as v_ps:
        for m in range(4):
            emit_qk(qk_ps, m)
        for si in range(NSC):
            ps = v_ps.tile([128, 260], f32)
            for k in range(KC):
                nc.tensor.matmul(
                    ps[:],
                    st.xt_t[k][:, si * 128:(si + 1) * 128],
                    st.wv_t[k][:],
                    start=(k == 0),
                    stop=(k == KC - 1 and not has_bv),
                )
            if has_bv:
                nc.tensor.matmul(
                    ps[:], st.ones_row[0:1, 0:128], st.bv_sb[:],
                    start=False, stop=True,
                )
            nc.scalar.copy(st.v_t[si][:], ps[:])
            for h in range(HPC):
                nc.gpsimd.memset(st.v_t[si][:, 65 * h + 64:65 * h + 65], 1.0)


def _attn_window(tc, nc, d, st, pools, p, qi8):
    """One qi window of 1024 for head-pair p: scores^T -> exp -> *exp(relT)
    -> PV accumulate -> normalize into attnT."""
    from concourse import mybir
    bf = mybir.dt.bfloat16
    f32 = mybir.dt.float32
    EXP = mybir.ActivationFunctionType.Exp
    (sc_ps, pv_ps, erb_pool, esc_pool, prob_pool, rec_pool, bc_pool) = pools

    qT = st.qkT_t[2 * p]
    kT = st.qkT_t[2 * p + 1]
    w0 = qi8 * 1024
    w1 = w0 + 1024
    nkj = w1 // 128
    # pv accumulators: [head][q4] -> [65, 512]
    pv = [[pv_ps.tile([65, 512], f32, name="pv", tag="pv") for _ in range(2)]
          for _ in range(2)]

    for kj in range(nkj):
        qs = max(w0, (kj * 128) // 512 * 512)
        width = w1 - qs
        # both heads' score MMs adjacent: consecutive MMs hit alternating
        # PE row groups, letting LDWEIGHTS overlap the in-flight matmul
        sc = [sc_ps.tile([128, width], f32, name="sc", tag="sc")
              for _ in range(2)]
        for off in range(qs, w1, 512):
            for h in range(2):
                rows = slice(64 * h, 64 * h + 64)
                nc.tensor.matmul(
                    sc[h][:, off - qs:off - qs + 512],
                    kT[rows, kj * 128:(kj + 1) * 128],
                    qT[rows, off:off + 512],
                    start=True, stop=True,
                    tile_position=(64 * h, 0),
                )
        pr = [None, None]
        for h in range(2):
            hl = 2 * p + h  # local head index
            esc = esc_pool.tile([128, width], bf, name="esc", tag="esc")
            nc.scalar.activation(esc[:], sc[h][:], EXP)
            rb = erb_pool.tile([128, width], bf, name="erb", tag="erb")
            nc.sync.dma_start(
                rb[:], d.erb[hl, kj * 128:(kj + 1) * 128, qs:w1])
            pr[h] = prob_pool.tile([128, width], bf, name="prob", tag="prob")
            nc.vector.tensor_mul(pr[h][:], esc[:], rb[:])
        for h in range(2):
            hl = 2 * p + h
            for off in range(qs, w1, 512):
                q4 = (off - w0) // 512
                last_kj = 8 * qi8 + 4 * q4 + 3
                nc.tensor.matmul(
                    pv[h][q4][:],
                    st.v_t[kj][:, 65 * hl:65 * hl + 65],
                    pr[h][:, off - qs:off - qs + 512],
                    start=(kj == 0),
                    stop=(kj == last_kj),
                )
    # normalization for this qi window: copy the denominator rows (PSUM
    # partition 64) to SBUF, broadcast both heads' rows across partitions
    # with two accumulated K=1 matmuls (sel rows pick partitions 0-63 /
    # 64-127), then one parallel reciprocal per q4 instead of single-lane
    # vector.reciprocal + a DRAM bounce.
    den = [rec_pool.tile([1, 1024], mybir.dt.float32r, name="rec", tag="rec")
           for _ in range(2)]
    for h in range(2):
        for q4 in range(2):
            nc.vector.tensor_copy(
                den[h][0:1, q4 * 512:(q4 + 1) * 512],
                pv[h][q4][64:65, :])
    for q4 in range(2):
        bc_ps = sc_ps.tile([128, 512], f32, name="sc", tag="sc")
        nc.tensor.matmul(
            bc_ps[:], st.sel_f32r[0:1, 0:128],
            den[0][0:1, q4 * 512:(q4 + 1) * 512], start=True, stop=False)
        nc.tensor.matmul(
            bc_ps[:], st.sel_f32r[0:1, 128:256],
            den[1][0:1, q4 * 512:(q4 + 1) * 512], start=False, stop=True)
        bc = bc_pool.tile([128, 512], f32, name="bc", tag="bc")
        nc.vector.reciprocal_approx_fast(bc[:], bc_ps[:])
        for h in range(2):
            nc.vector.tensor_mul(
                st.attnT_t[p][64 * h:64 * h + 64,
                              w0 + q4 * 512:w0 + (q4 + 1) * 512],
                pv[h][q4][0:64, :],
                bc[64 * h:64 * h + 64, :])


def _phase_attn(ctx, tc, nc, d, st):
    from concourse import mybir
    bf = mybir.dt.bfloat16

    attnT_pool = ctx.enter_context(tc.tile_pool(name="attnT", bufs=2))
    st.attnT_t = [attnT_pool.tile([128, S], bf, name="attnT", tag="attnT") for _ in range(2)]

    with ExitStack() as cctx:
        pools = (
            cctx.enter_context(tc.tile_pool(name="sc_ps", bufs=2, space="PSUM")),
            cctx.enter_context(tc.tile_pool(name="pv_ps", bufs=4, space="PSUM")),
            cctx.enter_context(tc.tile_pool(name="erb", bufs=10)),
            cctx.enter_context(tc.tile_pool(name="esc", bufs=8)),
            cctx.enter_context(tc.tile_pool(name="prob", bufs=8)),
            cctx.enter_context(tc.tile_pool(name="rec", bufs=4)),
            cctx.enter_context(tc.tile_pool(name="bc", bufs=3)),
        )
        for p in range(2):
            for qi8 in range(2):
                _attn_window(tc, nc, d, st, pools, p, qi8)


def _phase_out(ctx, tc, nc, d, st):
    from concourse import mybir
    f32 = mybir.dt.float32

    with tc.tile_pool(name="o_ps", bufs=2, space="PSUM") as o_ps, \
         tc.tile_pool(name="osb", bufs=4) as osb_pool:
        for si in range(NSC):
            for e2 in range(2):
                ps = o_ps.tile([128, 512], f32, name="ops", tag="ops")
                for p in range(2):
                    nc.tensor.matmul(
                        ps[:],
                        st.attnT_t[p][:, si * 128:(si + 1) * 128],
                        st.wo_t[p][:, e2 * 512:(e2 + 1) * 512],
                        start=(p == 0), stop=(p == 1),
                    )
                osb = osb_pool.tile([128, 512], f32, name="osb", tag="osb")
                if e2 == 0:
                    nc.vector.tensor_copy(osb[:], ps[:])
                else:
                    nc.scalar.copy(osb[:], ps[:])
                nc.sync.dma_start(
                    d.out[si * 128:(si + 1) * 128, e2 * 512:(e2 + 1) * 512],
                    osb[:])


_LDW_OPT_INSTALLED = False


def _enable_ldw_opt():
    """walrus ships with --enable-ldw-opt=false; flip it for this process
    (dedupes/hoists LDWEIGHTS). Gated by KERNEL_LDW_OPT=1."""
    global _LDW_OPT_INSTALLED
    if _LDW_OPT_INSTALLED:
        return
    _LDW_OPT_INSTALLED = True
    import os
    if os.environ.get("KERNEL_LDW_OPT", "0") != "1":
        return
    import concourse.bass_utils as bu
    orig = bu.run_command

    def patched(argv, **kwargs):
        argv = ["--enable-ldw-opt=true" if a == "--enable-ldw-opt=false" else a
                for a in argv]
        return orig(argv, **kwargs)

    bu.run_command = patched


def _build_program(has_bqk: bool, has_bv: bool):
    import concourse.tile as tile
    from concourse import bacc, mybir

    bf = mybir.dt.bfloat16
    f32 = mybir.dt.float32

    nc = bacc.Bacc("TRN2", target_bir_lowering=False, debug=False,
                   num_devices=NCORES)

    d = types.SimpleNamespace()
    d.xT = nc.dram_tensor("xT", [D, S], bf, kind="ExternalInput").ap()
    d.wqk = nc.dram_tensor("wqk", [D, 512], bf, kind="ExternalInput").ap()
    d.wv = nc.dram_tensor("wv", [D, 260], bf, kind="ExternalInput").ap()
    d.bqk = nc.dram_tensor("bqk", [4, 128], bf, kind="ExternalInput").ap()
    d.bv = nc.dram_tensor("bv", [1, 260], bf, kind="ExternalInput").ap()
    d.erb = nc.dram_tensor("erb", [HPC, S, S], bf, kind="ExternalInput").ap()
    d.wo = nc.dram_tensor("wo", [2, 128, D], bf, kind="ExternalInput").ap()
    d.out = nc.dram_tensor("out", [S, D], f32, kind="ExternalOutput").ap()

    st = types.SimpleNamespace()
    with tile.TileContext(nc) as tc:
        with ExitStack() as ctx:
            _phase_load(ctx, tc, nc, d, has_bqk, has_bv, st)
            _phase_proj(ctx, tc, nc, has_bqk, has_bv, st)
            _phase_attn(ctx, tc, nc, d, st)
            _phase_out(ctx, tc, nc, d, st)

    nc.compile()
    return nc


_PROGRAM_CACHE = {}


def _get_program(has_bqk, has_bv):
    key = (has_bqk, has_bv)
    if key not in _PROGRAM_CACHE:
        _PROGRAM_CACHE[key] = _build_program(has_bqk, has_bv)
    return _PROGRAM_CACHE[key]


_last_results = None  # BassKernelResults of the most recent run (for test.py)


def kernel(x, rel_bias, w_qkv, b_qkv, w_out, b_out, *, trace=False):
    global _last_results
    _install_ntff_hook()
    _enable_ldw_opt()
    from concourse.bass_utils import run_bass_kernel_spmd

    x = np.asarray(x, dtype=np.float32)
    rel_bias = np.asarray(rel_bias, dtype=np.float32)
    w_qkv = np.asarray(w_qkv, dtype=np.float32)
    b_qkv = np.asarray(b_qkv, dtype=np.float32)
    w_out = np.asarray(w_out, dtype=np.float32)
    b_out = np.asarray(b_out, dtype=np.float32)

    wq = w_qkv[:, 0:D]
    wk = w_qkv[:, D:2 * D]
    wv = w_qkv[:, 2 * D:3 * D]
    bq, bk, bv = b_qkv[0:D], b_qkv[D:2 * D], b_qkv[2 * D:3 * D]
    has_bqk = bool(np.any(bq)) or bool(np.any(bk))
    has_bv = bool(np.any(bv))

    nc = _get_program(has_bqk, has_bv)

    sc = 1.0 / math.sqrt(HD)  # folded into the q projection
    xT = [np.ascontiguousarray(x[b].T).astype(_BF16) for b in range(B)]
    tri = np.triu(np.ones((S, S), dtype=np.float32))  # [kj, qi]: qi >= kj

    in_maps = []
    for c in range(NCORES):
        b, hg = divmod(c, 4)
        hs = [4 * hg + i for i in range(HPC)]

        # wqk columns: [q_h0 | q_h1 | k_h0 | k_h1 | q_h2 | q_h3 | k_h2 | k_h3]
        cols = []
        bqk_rows = []
        for pair in range(2):
            h0, h1 = hs[2 * pair], hs[2 * pair + 1]
            cols += [wq[:, HD * h0:HD * (h0 + 1)] * sc,
                     wq[:, HD * h1:HD * (h1 + 1)] * sc]
            bqk_rows.append(np.concatenate(
                [bq[HD * h0:HD * (h0 + 1)], bq[HD * h1:HD * (h1 + 1)]]) * sc)
            cols += [wk[:, HD * h0:HD * (h0 + 1)],
                     wk[:, HD * h1:HD * (h1 + 1)]]
            bqk_rows.append(np.concatenate(
                [bk[HD * h0:HD * (h0 + 1)], bk[HD * h1:HD * (h1 + 1)]]))
        wqk_c = np.concatenate(cols, axis=1).astype(_BF16)
        bqk_c = np.stack(bqk_rows).astype(_BF16)

        wv_c = np.zeros((D, 260), dtype=np.float32)
        bv_c = np.zeros((1, 260), dtype=np.float32)
        for i, h in enumerate(hs):
            wv_c[:, 65 * i:65 * i + 64] = wv[:, HD * h:HD * (h + 1)]
            bv_c[0, 65 * i:65 * i + 64] = bv[HD * h:HD * (h + 1)]

        erb_c = np.empty((HPC, S, S), dtype=_BF16)
        for i, h in enumerate(hs):
            erb_c[i] = (np.exp(rel_bias[h].T) * tri).astype(_BF16)

        in_maps.append({
            "xT": xT[b],
            "wqk": wqk_c,
            "wv": wv_c.astype(_BF16),
            "bqk": bqk_c,
            "bv": bv_c.astype(_BF16),
            "erb": erb_c,
            "wo": np.ascontiguousarray(
                w_out[256 * hg:256 * (hg + 1)].reshape(2, 128, D)).astype(_BF16),
        })

    res = run_bass_kernel_spmd(nc, in_maps, list(range(NCORES)), trace=trace)
    _last_results = res

    out = np.zeros((B, S, D), dtype=np.float32)
    for c in range(NCORES):
        out[c // 4] += res.results[c]["out"]
    out += b_out
    return out



# revision 6
# speedup vs baseline: 1.0592x; 1.0592x over previous
"""Causal multi-head attention with relative position bias on 8 Trainium2
NeuronCores.

Problem (full shapes): x[2,2048,1024], rel_bias[16,2048,2048],
w_qkv[1024,3072], b_qkv[3072], w_out[1024,1024], b_out[1024].

Sharding: core = (batch, head-group): 2 batches x 4 head-groups of 4 heads.
Each core computes q/k/v projections for its 4 heads, causal attention with
rel-bias, and a partial output projection through its heads' rows of w_out.
Host sums the 4 partial outputs per batch (the tensor-parallel reduce) and
adds b_out.

Device kernel design notes:
- Scores are computed TRANSPOSED (scoresT[kj,qi] = k.q) so no on-chip
  transposes are needed anywhere: softmax reduction over keys becomes a
  matmul contraction, handled by appending a ones-column to V; the PV matmul
  directly produces the transposed attention output that the out-projection
  needs as its stationary operand.
- exp(score + bias) = exp(score) * exp(bias): host precomputes exp(rel_biasT)
  in bf16 with the causal mask baked in as exact zeros. ACT does a pure exp
  straight from PSUM; DVE multiplies two bf16 SBUF operands at 2x rate.
- All matmul operands are bf16 (PSUM accumulation is fp32); softmax
  denominators, reciprocals and the normalization are fp32.
- The per-query normalization 1/denom is broadcast across partitions with a
  stride-0 SBUF->SBUF DMA and applied to the small attention output, not the
  big probability matrix.
"""

import math
import sys
import types
from contextlib import ExitStack

import ml_dtypes
import numpy as np

B, S, D = 2, 2048, 1024
NH, HD = 16, 64
NCORES = 8
HPC = 4  # heads per core (2 pairs)

_BF16 = ml_dtypes.bfloat16


def _install_ntff_hook():
    """concourse.bass_utils imports antenv.axon_hooks for NTFF tracing under
    axon; this container's antenv lacks that module. Provide it, backed by
    the ctypes hook from trn_agent_boot (if present)."""
    if "antenv.axon_hooks" in sys.modules:
        return
    try:
        import antenv
    except ImportError:
        return
    mod = types.ModuleType("antenv.axon_hooks")
    mod._hook = None
    mod.set_axon_ntff_profile_hook = lambda h: setattr(mod, "_hook", h)
    mod.get_axon_ntff_profile_hook = lambda: mod._hook
    sys.modules["antenv.axon_hooks"] = mod
    antenv.axon_hooks = mod
    try:
        from trn_agent_boot.trn_boot import _ntff_profile_via_ctypes

        h = _ntff_profile_via_ctypes("/opt/axon/libaxon_pjrt.so")
        if h is not None:
            mod._hook = h
    except Exception:
        pass


KC = D // 128   # 8 contraction chunks for the projections
NS4 = S // 512  # 4 s-superblocks
NSC = S // 128  # 16 s-chunks


def _phase_load(ctx, tc, nc, d, has_bqk, has_bv, st):
    """DMA weights + xT into persistent SBUF tiles."""
    from concourse import mybir
    bf = mybir.dt.bfloat16

    xt_pool = ctx.enter_context(tc.tile_pool(name="xt", bufs=KC))
    wqk_pool = ctx.enter_context(tc.tile_pool(name="wqk", bufs=KC))
    wv_pool = ctx.enter_context(tc.tile_pool(name="wv", bufs=KC))
    wo_pool = ctx.enter_context(tc.tile_pool(name="wo", bufs=2))
    const_pool = ctx.enter_context(tc.tile_pool(name="consts", bufs=1))

    st.ones_row = const_pool.tile([1, 512], bf)
    nc.gpsimd.memset(st.ones_row[:], 1.0)
    # selection rows for the denominator broadcast: selA hits partitions
    # 0-63, selB partitions 64-127 (K=1 matmuls accumulate both)
    sel_f32 = const_pool.tile([1, 256], mybir.dt.float32)
    nc.gpsimd.memset(sel_f32[:], 0.0)
    nc.gpsimd.memset(sel_f32[0:1, 0:64], 1.0)
    nc.gpsimd.memset(sel_f32[0:1, 192:256], 1.0)
    st.sel_f32r = const_pool.tile([1, 256], mybir.dt.float32r)
    nc.vector.tensor_copy(st.sel_f32r[:], sel_f32[:])

    st.wqk_t, st.xt_t, st.wv_t = [], [], []
    for k in range(KC):
        w = wqk_pool.tile([128, 512], bf)
        nc.sync.dma_start(w[:], d.wqk[k * 128:(k + 1) * 128, :])
        st.wqk_t.append(w)
        xt = xt_pool.tile([128, S], bf)
        nc.sync.dma_start(xt[:], d.xT[k * 128:(k + 1) * 128, :])
        st.xt_t.append(xt)
    for k in range(KC):
        # wv is first consumed ~30us in; keep it out of the critical
        # DMA prefix that the first qk accumulation chain waits on
        wv = wv_pool.tile([128, 260], bf)
        nc.sync.dma_start(wv[:], d.wv[k * 128:(k + 1) * 128, :])
        st.wv_t.append(wv)
    st.wo_t = []
    for p in range(2):
        w = wo_pool.tile([128, D], bf)
        nc.sync.dma_start(w[:], d.wo[p])
        st.wo_t.append(w)
    if has_bqk:
        st.bqk_sb = []
        for m in range(4):
            t = const_pool.tile([1, 128], bf, name=f"bqk{m}", tag=f"bqk{m}")
            nc.sync.dma_start(t[:], d.bqk[m:m + 1, :])
            st.bqk_sb.append(t)
    if has_bv:
        st.bv_sb = const_pool.tile([1, 260], bf)
        nc.sync.dma_start(st.bv_sb[:], d.bv[:])


def _phase_proj(ctx, tc, nc, has_bqk, has_bv, st):
    """qkv projections.

    qkT[m][r, s]: m-chunks 0..3 = [q pair0 | k pair0 | q pair1 | k pair1];
    within a chunk rows 0-63 = first head of the pair, 64-127 = second.
    v_t[si]: [128, 260] bf16, 4 slots of 65 cols (64 v-cols + ones col).
    """
    from concourse import mybir
    bf = mybir.dt.bfloat16
    f32 = mybir.dt.float32

    qkT_pool = ctx.enter_context(tc.tile_pool(name="qkT", bufs=4))
    v_pool = ctx.enter_context(tc.tile_pool(name="vsb", bufs=NSC))
    st.qkT_t = [qkT_pool.tile([128, S], bf, name="qkT", tag="qkT") for _ in range(4)]
    st.v_t = [v_pool.tile([128, 260], bf, name="vsb", tag="vsb") for _ in range(NSC)]

    def emit_qk(qk_ps, m):
        for s4 in range(NS4):
            ps = qk_ps.tile([128, 512], f32, name="qkps", tag="qkps")
            for k in range(KC):
                nc.tensor.matmul(
                    ps[:],
                    st.wqk_t[k][:, m * 128:(m + 1) * 128],
                    st.xt_t[k][:, s4 * 512:(s4 + 1) * 512],
                    start=(k == 0),
                    stop=(k == KC - 1 and not has_bqk),
                )
            if has_bqk:
                nc.tensor.matmul(
                    ps[:], st.bqk_sb[m][:], st.ones_row[:, :],
                    start=False, stop=True,
                )
            nc.vector.tensor_copy(
                st.qkT_t[m][:, s4 * 512:(s4 + 1) * 512], ps[:])

    with tc.tile_pool(name="qk_ps", bufs=4, space="PSUM") as qk_ps, \
         tc.tile_pool(name="v_ps", bufs=3, space="PSUM") as v_ps:
        for m in range(4):
            emit_qk(qk_ps, m)
        for si in range(NSC):
            ps = v_ps.tile([128, 260], f32)
            for k in range(KC):
                nc.tensor.matmul(
                    ps[:],
                    st.xt_t[k][:, si * 128:(si + 1) * 128],
                    st.wv_t[k][:],
                    start=(k == 0),
                    stop=(k == KC - 1 and not has_bv),
                )
            if has_bv:
                nc.tensor.matmul(
                    ps[:], st.ones_row[0:1, 0:128], st.bv_sb[:],
                    start=False, stop=True,
                )
            nc.scalar.copy(st.v_t[si][:], ps[:])
            for h in range(HPC):
                nc.gpsimd.memset(st.v_t[si][:, 65 * h + 64:65 * h + 65], 1.0)


def _attn_window(tc, nc, d, st, pools, p, qi8):
    """One qi window of 1024 for head-pair p: scores^T -> exp -> *exp(relT)
    -> PV accumulate -> normalize into attnT."""
    from concourse import mybir
    bf = mybir.dt.bfloat16
    f32 = mybir.dt.float32
    EXP = mybir.ActivationFunctionType.Exp
    (sc_ps, pv_ps, erb_pool, esc_pool, prob_pool, rec_pool, bc_pool) = pools

    qT = st.qkT_t[2 * p]
    kT = st.qkT_t[2 * p + 1]
    w0 = qi8 * 1024
    w1 = w0 + 1024
    nkj = w1 // 128
    # pv accumulators: [head][q4] -> [65, 512]
    pv = [[pv_ps.tile([65, 512], f32, name="pv", tag="pv") for _ in range(2)]
          for _ in range(2)]

    for kj in range(nkj):
        qs = max(w0, (kj * 128) // 512 * 512)
        width = w1 - qs
        # both heads' score MMs adjacent: consecutive MMs hit alternating
        # PE row groups, letting LDWEIGHTS overlap the in-flight matmul
        sc = [sc_ps.tile([128, width], f32, name="sc", tag="sc")
              for _ in range(2)]
        for off in range(qs, w1, 512):
            for h in range(2):
                rows = slice(64 * h, 64 * h + 64)
                nc.tensor.matmul(
                    sc[h][:, off - qs:off - qs + 512],
                    kT[rows, kj * 128:(kj + 1) * 128],
                    qT[rows, off:off + 512],
                    start=True, stop=True,
                    tile_position=(64 * h, 0),
                )
        pr = [None, None]
        for h in range(2):
            hl = 2 * p + h  # local head index
            esc = esc_pool.tile([128, width], bf, name="esc", tag="esc")
            nc.scalar.activation(esc[:], sc[h][:], EXP)
            rb = erb_pool.tile([128, width], bf, name="erb", tag="erb")
            nc.sync.dma_start(
                rb[:], d.erb[hl, kj * 128:(kj + 1) * 128, qs:w1])
            pr[h] = prob_pool.tile([128, width], bf, name="prob", tag="prob")
            nc.vector.tensor_mul(pr[h][:], esc[:], rb[:])
        for h in range(2):
            hl = 2 * p + h
            for off in range(qs, w1, 512):
                q4 = (off - w0) // 512
                last_kj = 8 * qi8 + 4 * q4 + 3
                nc.tensor.matmul(
                    pv[h][q4][:],
                    st.v_t[kj][:, 65 * hl:65 * hl + 65],
                    pr[h][:, off - qs:off - qs + 512],
                    start=(kj == 0),
                    stop=(kj == last_kj),
                )
    # normalization for this qi window: copy the denominator rows (PSUM
    # partition 64) to SBUF, broadcast each head's row across its 64
    # partitions on the idle GpSimd engine (keeps TensorE out of the
    # dependency chain), then one parallel reciprocal per q4 instead of
    # single-lane vector.reciprocal + a DRAM bounce.
    den = [rec_pool.tile([1, 1024], f32, name="rec", tag="rec")
           for _ in range(2)]
    for h in range(2):
        for q4 in range(2):
            nc.vector.tensor_copy(
                den[h][0:1, q4 * 512:(q4 + 1) * 512],
                pv[h][q4][64:65, :])
    for q4 in range(2):
        bc = [None, None]
        for h in range(2):
            bcd = bc_pool.tile([64, 512], f32, name=f"bcd{h}", tag=f"bcd{h}")
            nc.gpsimd.partition_broadcast(
                bcd[:], den[h][0:1, q4 * 512:(q4 + 1) * 512])
            bc[h] = bc_pool.tile([64, 512], f32, name=f"bc{h}", tag=f"bc{h}")
            nc.vector.reciprocal_approx_fast(bc[h][:], bcd[:])
        for h in range(2):
            nc.vector.tensor_mul(
                st.attnT_t[p][64 * h:64 * h + 64,
                              w0 + q4 * 512:w0 + (q4 + 1) * 512],
                pv[h][q4][0:64, :],
                bc[h][:])


def _phase_attn(ctx, tc, nc, d, st):
    from concourse import mybir
    bf = mybir.dt.bfloat16

    attnT_pool = ctx.enter_context(tc.tile_pool(name="attnT", bufs=2))
    st.attnT_t = [attnT_pool.tile([128, S], bf, name="attnT", tag="attnT") for _ in range(2)]

    with ExitStack() as cctx:
        pools = (
            cctx.enter_context(tc.tile_pool(name="sc_ps", bufs=2, space="PSUM")),
            cctx.enter_context(tc.tile_pool(name="pv_ps", bufs=4, space="PSUM")),
            cctx.enter_context(tc.tile_pool(name="erb", bufs=10)),
            cctx.enter_context(tc.tile_pool(name="esc", bufs=8)),
            cctx.enter_context(tc.tile_pool(name="prob", bufs=8)),
            cctx.enter_context(tc.tile_pool(name="rec", bufs=4)),
            cctx.enter_context(tc.tile_pool(name="bc", bufs=3)),
        )
        for p in range(2):
            for qi8 in range(2):
                _attn_window(tc, nc, d, st, pools, p, qi8)


def _phase_out(ctx, tc, nc, d, st):
    from concourse import mybir
    f32 = mybir.dt.float32

    with tc.tile_pool(name="o_ps", bufs=2, space="PSUM") as o_ps, \
         tc.tile_pool(name="osb", bufs=4) as osb_pool:
        for si in range(NSC):
            for e2 in range(2):
                ps = o_ps.tile([128, 512], f32, name="ops", tag="ops")
                for p in range(2):
                    nc.tensor.matmul(
                        ps[:],
                        st.attnT_t[p][:, si * 128:(si + 1) * 128],
                        st.wo_t[p][:, e2 * 512:(e2 + 1) * 512],
                        start=(p == 0), stop=(p == 1),
                    )
                osb = osb_pool.tile([128, 512], f32, name="osb", tag="osb")
                if e2 == 0:
                    nc.vector.tensor_copy(osb[:], ps[:])
                else:
                    nc.scalar.copy(osb[:], ps[:])
                nc.sync.dma_start(
                    d.out[si * 128:(si + 1) * 128, e2 * 512:(e2 + 1) * 512],
                    osb[:])


_LDW_OPT_INSTALLED = False


def _enable_ldw_opt():
    """walrus ships with --enable-ldw-opt=false; flip it for this process
    (dedupes/hoists LDWEIGHTS). Gated by KERNEL_LDW_OPT=1."""
    global _LDW_OPT_INSTALLED
    if _LDW_OPT_INSTALLED:
        return
    _LDW_OPT_INSTALLED = True
    import os
    if os.environ.get("KERNEL_LDW_OPT", "0") != "1":
        return
    import concourse.bass_utils as bu
    orig = bu.run_command

    def patched(argv, **kwargs):
        argv = ["--enable-ldw-opt=true" if a == "--enable-ldw-opt=false" else a
                for a in argv]
        return orig(argv, **kwargs)

    bu.run_command = patched


def _build_program(has_bqk: bool, has_bv: bool):
    import concourse.tile as tile
    from concourse import bacc, mybir

    bf = mybir.dt.bfloat16
    f32 = mybir.dt.float32

    nc = bacc.Bacc("TRN2", target_bir_lowering=False, debug=False,
                   num_devices=NCORES)

    d = types.SimpleNamespace()
    d.xT = nc.dram_tensor("xT", [D, S], bf, kind="ExternalInput").ap()
    d.wqk = nc.dram_tensor("wqk", [D, 512], bf, kind="ExternalInput").ap()
    d.wv = nc.dram_tensor("wv", [D, 260], bf, kind="ExternalInput").ap()
    d.bqk = nc.dram_tensor("bqk", [4, 128], bf, kind="ExternalInput").ap()
    d.bv = nc.dram_tensor("bv", [1, 260], bf, kind="ExternalInput").ap()
    d.erb = nc.dram_tensor("erb", [HPC, S, S], bf, kind="ExternalInput").ap()
    d.wo = nc.dram_tensor("wo", [2, 128, D], bf, kind="ExternalInput").ap()
    d.out = nc.dram_tensor("out", [S, D], f32, kind="ExternalOutput").ap()

    st = types.SimpleNamespace()
    with tile.TileContext(nc) as tc:
        with ExitStack() as ctx:
            _phase_load(ctx, tc, nc, d, has_bqk, has_bv, st)
            _phase_proj(ctx, tc, nc, has_bqk, has_bv, st)
            _phase_attn(ctx, tc, nc, d, st)
            _phase_out(ctx, tc, nc, d, st)

    nc.compile()
    return nc


_PROGRAM_CACHE = {}


def _get_program(has_bqk, has_bv):
    key = (has_bqk, has_bv)
    if key not in _PROGRAM_CACHE:
        _PROGRAM_CACHE[key] = _build_program(has_bqk, has_bv)
    return _PROGRAM_CACHE[key]


_last_results = None  # BassKernelResults of the most recent run (for test.py)


def kernel(x, rel_bias, w_qkv, b_qkv, w_out, b_out, *, trace=False):
    global _last_results
    _install_ntff_hook()
    _enable_ldw_opt()
    from concourse.bass_utils import run_bass_kernel_spmd

    x = np.asarray(x, dtype=np.float32)
    rel_bias = np.asarray(rel_bias, dtype=np.float32)
    w_qkv = np.asarray(w_qkv, dtype=np.float32)
    b_qkv = np.asarray(b_qkv, dtype=np.float32)
    w_out = np.asarray(w_out, dtype=np.float32)
    b_out = np.asarray(b_out, dtype=np.float32)

    wq = w_qkv[:, 0:D]
    wk = w_qkv[:, D:2 * D]
    wv = w_qkv[:, 2 * D:3 * D]
    bq, bk, bv = b_qkv[0:D], b_qkv[D:2 * D], b_qkv[2 * D:3 * D]
    has_bqk = bool(np.any(bq)) or bool(np.any(bk))
    has_bv = bool(np.any(bv))

    nc = _get_program(has_bqk, has_bv)

    sc = 1.0 / math.sqrt(HD)  # folded into the q projection
    xT = [np.ascontiguousarray(x[b].T).astype(_BF16) for b in range(B)]
    tri = np.triu(np.ones((S, S), dtype=np.float32))  # [kj, qi]: qi >= kj

    in_maps = []
    for c in range(NCORES):
        b, hg = divmod(c, 4)
        hs = [4 * hg + i for i in range(HPC)]

        # wqk columns: [q_h0 | q_h1 | k_h0 | k_h1 | q_h2 | q_h3 | k_h2 | k_h3]
        cols = []
        bqk_rows = []
        for pair in range(2):
            h0, h1 = hs[2 * pair], hs[2 * pair + 1]
            cols += [wq[:, HD * h0:HD * (h0 + 1)] * sc,
                     wq[:, HD * h1:HD * (h1 + 1)] * sc]
            bqk_rows.append(np.concatenate(
                [bq[HD * h0:HD * (h0 + 1)], bq[HD * h1:HD * (h1 + 1)]]) * sc)
            cols += [wk[:, HD * h0:HD * (h0 + 1)],
                     wk[:, HD * h1:HD * (h1 + 1)]]
            bqk_rows.append(np.concatenate(
                [bk[HD * h0:HD * (h0 + 1)], bk[HD * h1:HD * (h1 + 1)]]))
        wqk_c = np.concatenate(cols, axis=1).astype(_BF16)
        bqk_c = np.stack(bqk_rows).astype(_BF16)

        wv_c = np.zeros((D, 260), dtype=np.float32)
        bv_c = np.zeros((1, 260), dtype=np.float32)
        for i, h in enumerate(hs):
            wv_c[:, 65 * i:65 * i + 64] = wv[:, HD * h:HD * (h + 1)]
            bv_c[0, 65 * i:65 * i + 64] = bv[HD * h:HD * (h + 1)]

        erb_c = np.empty((HPC, S, S), dtype=_BF16)
        for i, h in enumerate(hs):
            erb_c[i] = (np.exp(rel_bias[h].T) * tri).astype(_BF16)

        in_maps.append({
            "xT": xT[b],
            "wqk": wqk_c,
            "wv": wv_c.astype(_BF16),
            "bqk": bqk_c,
            "bv": bv_c.astype(_BF16),
            "erb": erb_c,
            "wo": np.ascontiguousarray(
                w_out[256 * hg:256 * (hg + 1)].reshape(2, 128, D)).astype(_BF16),
        })

    res = run_bass_kernel_spmd(nc, in_maps, list(range(NCORES)), trace=trace)
    _last_results = res

    out = np.zeros((B, S, D), dtype=np.float32)
    for c in range(NCORES):
        out[c // 4] += res.results[c]["out"]
    out += b_out
    return out



# revision 7
# speedup vs baseline: 1.1111x; 1.0489x over previous
"""Causal multi-head attention with relative position bias on 8 Trainium2
NeuronCores.

Problem (full shapes): x[2,2048,1024], rel_bias[16,2048,2048],
w_qkv[1024,3072], b_qkv[3072], w_out[1024,1024], b_out[1024].

Sharding: core = (batch, head-group): 2 batches x 4 head-groups of 4 heads.
Each core computes q/k/v projections for its 4 heads, causal attention with
rel-bias, and a partial output projection through its heads' rows of w_out.
Host sums the 4 partial outputs per batch (the tensor-parallel reduce) and
adds b_out.

Device kernel design notes:
- Scores are computed TRANSPOSED (scoresT[kj,qi] = k.q) so no on-chip
  transposes are needed anywhere: softmax reduction over keys becomes a
  matmul contraction, handled by appending a ones-column to V; the PV matmul
  directly produces the transposed attention output that the out-projection
  needs as its stationary operand.
- exp(score + bias) = exp(score) * exp(bias): host precomputes exp(rel_biasT)
  in bf16 with the causal mask baked in as exact zeros. ACT does a pure exp
  straight from PSUM; DVE multiplies two bf16 SBUF operands at 2x rate.
- The whole kernel is emitted as ONE software-pipelined instruction stream:
  the attention inner loop is ACT(exp)-limited, so the ACT-free matmul work
  (qk/v projections, out projection) is interleaved into the attention
  stream as "filler" units.  This keeps the PE continuously busy, which
  matters twice: engine idle time, and the PE p-state ramp (the PE only
  reaches 2.4 GHz after ~3us of continuous execution; gaps drop it to
  1.2 GHz).
- Scores MMs for the two heads of a pair use tile_position row-tiling
  ((0,0)/(64,0)) so the K=64 matmuls execute concurrently.
- Normalization: denominator rows (PSUM partition 64 of the PV accumulator)
  are copied to SBUF, partition-broadcast on the idle GpSimd engine, and
  inverted with the fast all-lane reciprocal_approx_fast - never a
  single-lane vector.reciprocal, and nothing on the TensorE critical path.
- PSUM budget (8 banks): 4 PV accumulators + 3 score tiles + 1 shared
  filler bank.
"""

import math
import sys
import types
from contextlib import ExitStack

import ml_dtypes
import numpy as np

B, S, D = 2, 2048, 1024
NH, HD = 16, 64
NCORES = 8
HPC = 4  # heads per core (2 pairs)

_BF16 = ml_dtypes.bfloat16

KC = D // 128   # 8 contraction chunks for the projections
NS4 = S // 512  # 4 s-superblocks
NSC = S // 128  # 16 s-chunks


def _install_ntff_hook():
    """concourse.bass_utils imports antenv.axon_hooks for NTFF tracing under
    axon; this container's antenv lacks that module. Provide it, backed by
    the ctypes hook from trn_agent_boot (if present)."""
    if "antenv.axon_hooks" in sys.modules:
        return
    try:
        import antenv
    except ImportError:
        return
    mod = types.ModuleType("antenv.axon_hooks")
    mod._hook = None
    mod.set_axon_ntff_profile_hook = lambda h: setattr(mod, "_hook", h)
    mod.get_axon_ntff_profile_hook = lambda: mod._hook
    sys.modules["antenv.axon_hooks"] = mod
    antenv.axon_hooks = mod
    try:
        from trn_agent_boot.trn_boot import _ntff_profile_via_ctypes

        h = _ntff_profile_via_ctypes("/opt/axon/libaxon_pjrt.so")
        if h is not None:
            mod._hook = h
    except Exception:
        pass


def _build_program(has_bqk: bool, has_bv: bool):
    import concourse.tile as tile
    from concourse import bacc, mybir

    bf = mybir.dt.bfloat16
    f32 = mybir.dt.float32
    EXP = mybir.ActivationFunctionType.Exp

    nc = bacc.Bacc("TRN2", target_bir_lowering=False, debug=False,
                   num_devices=NCORES)

    d = types.SimpleNamespace()
    d.xT = nc.dram_tensor("xT", [D, S], bf, kind="ExternalInput").ap()
    d.wqk = nc.dram_tensor("wqk", [D, 512], bf, kind="ExternalInput").ap()
    d.wv = nc.dram_tensor("wv", [D, 260], bf, kind="ExternalInput").ap()
    d.bqk = nc.dram_tensor("bqk", [4, 128], bf, kind="ExternalInput").ap()
    d.bv = nc.dram_tensor("bv", [1, 260], bf, kind="ExternalInput").ap()
    d.erb = nc.dram_tensor("erb", [HPC, S, S], bf, kind="ExternalInput").ap()
    d.wo = nc.dram_tensor("wo", [2, 128, D], bf, kind="ExternalInput").ap()
    d.out = nc.dram_tensor("out", [S, D], f32, kind="ExternalOutput").ap()

    st = types.SimpleNamespace()
    with tile.TileContext(nc) as tc:
        with ExitStack() as ctx:
            ep = ctx.enter_context
            # --- pools -------------------------------------------------
            xt_pool = ep(tc.tile_pool(name="xt", bufs=KC))
            wqk_pool = ep(tc.tile_pool(name="wqk", bufs=KC))
            wv_pool = ep(tc.tile_pool(name="wv", bufs=KC))
            wo_pool = ep(tc.tile_pool(name="wo", bufs=2))
            const_pool = ep(tc.tile_pool(name="consts", bufs=1))
            qkT_pool = ep(tc.tile_pool(name="qkT", bufs=4))
            v_pool = ep(tc.tile_pool(name="vsb", bufs=NSC))
            attnT_pool = ep(tc.tile_pool(name="attnT", bufs=2))
            esc_pool = ep(tc.tile_pool(name="esc", bufs=6))
            erb_pool = ep(tc.tile_pool(name="erb", bufs=10))
            pr_pool = ep(tc.tile_pool(name="prob", bufs=6))
            den_pool = ep(tc.tile_pool(name="den", bufs=3))
            bc_pool = ep(tc.tile_pool(name="bc", bufs=2))
            osb_pool = ep(tc.tile_pool(name="osb", bufs=4))
            fill_ps = ep(tc.tile_pool(name="fill_ps", bufs=1, space="PSUM"))
            sc_ps = ep(tc.tile_pool(name="sc_ps", bufs=3, space="PSUM"))
            pv_ps = ep(tc.tile_pool(name="pv_ps", bufs=4, space="PSUM"))

            # --- consts ------------------------------------------------
            st.ones_row = const_pool.tile([1, 512], bf)
            nc.gpsimd.memset(st.ones_row[:], 1.0)

            # --- loads (emission order = rough completion order) -------
            st.wqk_t, st.xt_t, st.wv_t = [], [], []
            for k in range(KC):
                w = wqk_pool.tile([128, 512], bf)
                nc.sync.dma_start(w[:], d.wqk[k * 128:(k + 1) * 128, :])
                st.wqk_t.append(w)
                xt = xt_pool.tile([128, S], bf)
                nc.sync.dma_start(xt[:], d.xT[k * 128:(k + 1) * 128, :])
                st.xt_t.append(xt)
            for k in range(KC):
                wv = wv_pool.tile([128, 260], bf)
                nc.sync.dma_start(wv[:], d.wv[k * 128:(k + 1) * 128, :])
                st.wv_t.append(wv)
            st.wo_t = []
            for p in range(2):
                w = wo_pool.tile([128, D], bf)
                nc.sync.dma_start(w[:], d.wo[p])
                st.wo_t.append(w)
            if has_bqk:
                st.bqk_sb = []
                for m in range(4):
                    t = const_pool.tile([1, 128], bf, name=f"bqk{m}",
                                        tag=f"bqk{m}")
                    nc.sync.dma_start(t[:], d.bqk[m:m + 1, :])
                    st.bqk_sb.append(t)
            if has_bv:
                st.bv_sb = const_pool.tile([1, 260], bf)
                nc.sync.dma_start(st.bv_sb[:], d.bv[:])

            # --- persistent result tiles -------------------------------
            st.qkT_t = [qkT_pool.tile([128, S], bf, name="qkT", tag="qkT")
                        for _ in range(4)]
            st.v_t = [v_pool.tile([128, 260], bf, name="vsb", tag="vsb")
                      for _ in range(NSC)]
            st.attnT_t = [attnT_pool.tile([128, S], bf, name="attnT",
                                          tag="attnT") for _ in range(2)]

            # --- filler units ------------------------------------------
            def qk_unit(m, s4):
                """One qk-projection chain: qkT[m][:, s4*512:...]."""
                ps = fill_ps.tile([128, 512], f32, name="fps", tag="fps")
                for k in range(KC):
                    nc.tensor.matmul(
                        ps[:],
                        st.wqk_t[k][:, m * 128:(m + 1) * 128],
                        st.xt_t[k][:, s4 * 512:(s4 + 1) * 512],
                        start=(k == 0),
                        stop=(k == KC - 1 and not has_bqk),
                    )
                if has_bqk:
                    nc.tensor.matmul(
                        ps[:], st.bqk_sb[m][:], st.ones_row[:, :],
                        start=False, stop=True,
                    )
                nc.vector.tensor_copy(
                    st.qkT_t[m][:, s4 * 512:(s4 + 1) * 512], ps[:])

            def v_unit(si):
                """One v-projection chain: v_t[si] (4x 64 v-cols + ones)."""
                ps = fill_ps.tile([128, 512], f32, name="fps", tag="fps")
                for k in range(KC):
                    nc.tensor.matmul(
                        ps[:, 0:260],
                        st.xt_t[k][:, si * 128:(si + 1) * 128],
                        st.wv_t[k][:],
                        start=(k == 0),
                        stop=(k == KC - 1 and not has_bv),
                    )
                if has_bv:
                    nc.tensor.matmul(
                        ps[:, 0:260], st.ones_row[0:1, 0:128], st.bv_sb[:],
                        start=False, stop=True,
                    )
                nc.vector.tensor_copy(st.v_t[si][:], ps[:, 0:260])
                for h in range(HPC):
                    nc.gpsimd.memset(
                        st.v_t[si][:, 65 * h + 64:65 * h + 65], 1.0)

            def out_unit(si, e2):
                """One out-projection tile: out[si*128:.., e2*512:..]."""
                ps = fill_ps.tile([128, 512], f32, name="fps", tag="fps")
                for p in range(2):
                    nc.tensor.matmul(
                        ps[:],
                        st.attnT_t[p][:, si * 128:(si + 1) * 128],
                        st.wo_t[p][:, e2 * 512:(e2 + 1) * 512],
                        start=(p == 0), stop=(p == 1),
                    )
                osb = osb_pool.tile([128, 512], f32, name="osb", tag="osb")
                if e2 == 0:
                    nc.vector.tensor_copy(osb[:], ps[:])
                else:
                    nc.scalar.copy(osb[:], ps[:])
                nc.sync.dma_start(
                    d.out[si * 128:(si + 1) * 128,
                          e2 * 512:(e2 + 1) * 512],
                    osb[:])

            # --- one attention window (p = head pair, qi8 = q half) ----
            def window(p, qi8, fills):
                """fills: list of (min_unit_idx, emit_fn), FIFO order."""
                w0 = qi8 * 1024
                w1 = w0 + 1024
                nkj = w1 // 128
                units = [(kj, off) for kj in range(nkj)
                         for off in range(max(w0, (kj * 128) // 512 * 512),
                                          w1, 512)]
                n = len(units)
                pv = [[pv_ps.tile([65, 512], f32, name="pv", tag="pv")
                       for _ in range(2)] for _ in range(2)]
                erb_t = {}
                pr_t = {}

                def emit_erb(i):
                    kj, off = units[i]
                    ts = []
                    for h in range(2):
                        hl = 2 * p + h
                        rb = erb_pool.tile([128, 512], bf, name="erb",
                                           tag="erb")
                        nc.sync.dma_start(
                            rb[:],
                            d.erb[hl, kj * 128:(kj + 1) * 128, off:off + 512])
                        ts.append(rb)
                    erb_t[i] = ts

                def emit_sem(i):
                    kj, off = units[i]
                    qT = st.qkT_t[2 * p]
                    kT = st.qkT_t[2 * p + 1]
                    sc = [sc_ps.tile([128, 512], f32, name="sc", tag="sc")
                          for _ in range(2)]
                    for h in range(2):
                        rows = slice(64 * h, 64 * h + 64)
                        nc.tensor.matmul(
                            sc[h][:],
                            kT[rows, kj * 128:(kj + 1) * 128],
                            qT[rows, off:off + 512],
                            start=True, stop=True,
                            tile_position=(64 * h, 0),
                        )
                    prs = []
                    for h in range(2):
                        esc = esc_pool.tile([128, 512], bf, name="esc",
                                            tag="esc")
                        nc.scalar.activation(esc[:], sc[h][:], EXP)
                        pr = pr_pool.tile([128, 512], bf, name="prob",
                                          tag="prob")
                        nc.vector.tensor_mul(pr[:], esc[:], erb_t[i][h][:])
                        prs.append(pr)
                    del erb_t[i]
                    pr_t[i] = prs

                def emit_norm(q4):
                    den = []
                    for h in range(2):
                        t = den_pool.tile([1, 512], f32, name=f"den{h}",
                                          tag=f"den{h}")
                        nc.vector.tensor_copy(t[:], pv[h][q4][64:65, :])
                        den.append(t)
                    for h in range(2):
                        bcd = bc_pool.tile([64, 512], f32, name=f"bcd{h}",
                                           tag=f"bcd{h}")
                        nc.gpsimd.partition_broadcast(bcd[:], den[h][:])
                        bcr = bc_pool.tile([64, 512], f32, name=f"bc{h}",
                                           tag=f"bc{h}")
                        nc.vector.reciprocal_approx_fast(bcr[:], bcd[:])
                        nc.vector.tensor_mul(
                            st.attnT_t[p][64 * h:64 * h + 64,
                                          w0 + q4 * 512:w0 + (q4 + 1) * 512],
                            pv[h][q4][0:64, :],
                            bcr[:])

                def emit_pv(i):
                    kj, off = units[i]
                    q4 = (off - w0) // 512
                    last_kj = 8 * qi8 + 4 * q4 + 3
                    for h in range(2):
                        hl = 2 * p + h
                        nc.tensor.matmul(
                            pv[h][q4][:],
                            st.v_t[kj][:, 65 * hl:65 * hl + 65],
                            pr_t[i][h][:],
                            start=(kj == 0),
                            stop=(kj == last_kj),
                        )
                    del pr_t[i]
                    if kj == last_kj:
                        emit_norm(q4)

                PF = 3  # erb prefetch distance in units
                for i in range(min(PF, n)):
                    emit_erb(i)
                emit_sem(0)
                nf = len(fills)
                fi = 0
                for i in range(n):
                    if i + PF < n:
                        emit_erb(i + PF)
                    if i + 1 < n:
                        emit_sem(i + 1)
                    target = (nf * (i + 1)) // n
                    while (fi < nf and fi < target
                           and fills[fi][0] <= i):
                        fills[fi][1]()
                        fi += 1
                    emit_pv(i)
                while fi < nf:
                    fills[fi][1]()
                    fi += 1

            # --- the schedule ------------------------------------------
            def F(fn, *a):
                return (0, (lambda: fn(*a)))

            # pre-phase: the minimum needed for window (p0, q0) kj0.
            qk_unit(0, 0)
            qk_unit(0, 1)
            qk_unit(1, 0)
            v_unit(0)

            window(0, 0, [
                F(v_unit, 1), F(v_unit, 2), F(qk_unit, 1, 1),
                F(v_unit, 3), F(qk_unit, 2, 0), F(v_unit, 4),
                F(qk_unit, 2, 1), F(v_unit, 5), F(qk_unit, 3, 0),
                F(v_unit, 6), F(v_unit, 7),
            ])
            window(1, 0, [
                F(qk_unit, 3, 1), F(qk_unit, 0, 2), F(v_unit, 8),
                F(qk_unit, 0, 3), F(v_unit, 9), F(v_unit, 10),
                F(v_unit, 11), F(v_unit, 12), F(v_unit, 13),
            ])
            w2_fills = [
                F(qk_unit, 1, 2), F(qk_unit, 2, 2), F(qk_unit, 1, 3),
                F(qk_unit, 2, 3), F(qk_unit, 3, 2), F(qk_unit, 3, 3),
                F(v_unit, 14), F(v_unit, 15),
            ]
            for si in range(0, 4):
                for e2 in range(2):
                    w2_fills.append((2, (lambda si=si, e2=e2:
                                         out_unit(si, e2))))
            window(0, 1, w2_fills)
            w3_fills = []
            for si in range(4, 8):
                for e2 in range(2):
                    w3_fills.append((0, (lambda si=si, e2=e2:
                                         out_unit(si, e2))))
            for si in range(8, 12):
                for e2 in range(2):
                    w3_fills.append((24, (lambda si=si, e2=e2:
                                          out_unit(si, e2))))
            window(1, 1, w3_fills)
            for si in range(12, NSC):
                for e2 in range(2):
                    out_unit(si, e2)

    nc.compile()
    return nc


_PROGRAM_CACHE = {}


def _get_program(has_bqk, has_bv):
    key = (has_bqk, has_bv)
    if key not in _PROGRAM_CACHE:
        _PROGRAM_CACHE[key] = _build_program(has_bqk, has_bv)
    return _PROGRAM_CACHE[key]


_last_results = None  # BassKernelResults of the most recent run (for test.py)


def kernel(x, rel_bias, w_qkv, b_qkv, w_out, b_out, *, trace=False):
    global _last_results
    _install_ntff_hook()
    from concourse.bass_utils import run_bass_kernel_spmd

    x = np.asarray(x, dtype=np.float32)
    rel_bias = np.asarray(rel_bias, dtype=np.float32)
    w_qkv = np.asarray(w_qkv, dtype=np.float32)
    b_qkv = np.asarray(b_qkv, dtype=np.float32)
    w_out = np.asarray(w_out, dtype=np.float32)
    b_out = np.asarray(b_out, dtype=np.float32)

    wq = w_qkv[:, 0:D]
    wk = w_qkv[:, D:2 * D]
    wv = w_qkv[:, 2 * D:3 * D]
    bq, bk, bv = b_qkv[0:D], b_qkv[D:2 * D], b_qkv[2 * D:3 * D]
    has_bqk = bool(np.any(bq)) or bool(np.any(bk))
    has_bv = bool(np.any(bv))

    nc = _get_program(has_bqk, has_bv)

    sc = 1.0 / math.sqrt(HD)  # folded into the q projection
    xT = [np.ascontiguousarray(x[b].T).astype(_BF16) for b in range(B)]
    tri = np.triu(np.ones((S, S), dtype=np.float32))  # [kj, qi]: qi >= kj

    in_maps = []
    for c in range(NCORES):
        b, hg = divmod(c, 4)
        hs = [4 * hg + i for i in range(HPC)]

        # wqk columns: [q_h0 | q_h1 | k_h0 | k_h1 | q_h2 | q_h3 | k_h2 | k_h3]
        cols = []
        bqk_rows = []
        for pair in range(2):
            h0, h1 = hs[2 * pair], hs[2 * pair + 1]
            cols += [wq[:, HD * h0:HD * (h0 + 1)] * sc,
                     wq[:, HD * h1:HD * (h1 + 1)] * sc]
            bqk_rows.append(np.concatenate(
                [bq[HD * h0:HD * (h0 + 1)], bq[HD * h1:HD * (h1 + 1)]]) * sc)
            cols += [wk[:, HD * h0:HD * (h0 + 1)],
                     wk[:, HD * h1:HD * (h1 + 1)]]
            bqk_rows.append(np.concatenate(
                [bk[HD * h0:HD * (h0 + 1)], bk[HD * h1:HD * (h1 + 1)]]))
        wqk_c = np.concatenate(cols, axis=1).astype(_BF16)
        bqk_c = np.stack(bqk_rows).astype(_BF16)

        wv_c = np.zeros((D, 260), dtype=np.float32)
        bv_c = np.zeros((1, 260), dtype=np.float32)
        for i, h in enumerate(hs):
            wv_c[:, 65 * i:65 * i + 64] = wv[:, HD * h:HD * (h + 1)]
            bv_c[0, 65 * i:65 * i + 64] = bv[HD * h:HD * (h + 1)]

        erb_c = np.empty((HPC, S, S), dtype=_BF16)
        for i, h in enumerate(hs):
            erb_c[i] = (np.exp(rel_bias[h].T) * tri).astype(_BF16)

        in_maps.append({
            "xT": xT[b],
            "wqk": wqk_c,
            "wv": wv_c.astype(_BF16),
            "bqk": bqk_c,
            "bv": bv_c.astype(_BF16),
            "erb": erb_c,
            "wo": np.ascontiguousarray(
                w_out[256 * hg:256 * (hg + 1)].reshape(2, 128, D)).astype(_BF16),
        })

    res = run_bass_kernel_spmd(nc, in_maps, list(range(NCORES)), trace=trace)
    _last_results = res

    out = np.zeros((B, S, D), dtype=np.float32)
    for c in range(NCORES):
        out[c // 4] += res.results[c]["out"]
    out += b_out
    return out


# revision 10
# speedup vs baseline: 1.1699x; 1.0530x over previous
"""Causal multi-head attention with relative position bias on 8 Trainium2
NeuronCores.

Problem (full shapes): x[2,2048,1024], rel_bias[16,2048,2048],
w_qkv[1024,3072], b_qkv[3072], w_out[1024,1024], b_out[1024].

Sharding: core = (batch, head-group): 2 batches x 4 head-groups of 4 heads.
Each core computes q/k/v projections for its 4 heads, causal attention with
rel-bias, and a partial output projection through its heads' rows of w_out.
Host sums the 4 partial outputs per batch (the tensor-parallel reduce) and
adds b_out.

Device kernel design notes:
- Scores are computed TRANSPOSED (scoresT[kj,qi] = k.q) so no on-chip
  transposes are needed anywhere: softmax reduction over keys becomes a
  matmul contraction, handled by appending a ones-column to V; the PV matmul
  directly produces the transposed attention output that the out-projection
  needs as its stationary operand.
- exp(score + bias) = exp(score) * exp(bias): host precomputes exp(rel_biasT)
  in bf16 with the causal mask baked in as exact zeros. ACT does a pure exp
  straight from PSUM; DVE multiplies two bf16 SBUF operands at 2x rate.
- The whole kernel is emitted as ONE software-pipelined instruction stream:
  the attention inner loop is ACT(exp)-limited, so the ACT-free matmul work
  (qk/v projections, out projection) is interleaved into the attention
  stream as "filler" units.  This keeps the PE continuously busy, which
  matters twice: engine idle time, and the PE p-state ramp (the PE only
  reaches 2.4 GHz after ~3us of continuous execution; gaps drop it to
  1.2 GHz).
- Scores MMs for the two heads of a pair use tile_position row-tiling
  ((0,0)/(64,0)) so the K=64 matmuls execute concurrently.
- Normalization: denominator rows (PSUM partition 64 of the PV accumulator)
  are copied to SBUF, partition-broadcast on the idle GpSimd engine, and
  inverted with the fast all-lane reciprocal_approx_fast - never a
  single-lane vector.reciprocal, and nothing on the TensorE critical path.
- PSUM budget (8 banks): 4 PV accumulators + 3 score tiles + 1 shared
  filler bank.
"""

import math
import sys
import types
from contextlib import ExitStack

import ml_dtypes
import numpy as np

B, S, D = 2, 2048, 1024
NH, HD = 16, 64
NCORES = 8
HPC = 4  # heads per core (2 pairs)

_BF16 = ml_dtypes.bfloat16

KC = D // 128   # 8 contraction chunks for the projections
NS4 = S // 512  # 4 s-superblocks
NSC = S // 128  # 16 s-chunks


def _install_ntff_hook():
    """concourse.bass_utils imports antenv.axon_hooks for NTFF tracing under
    axon; this container's antenv lacks that module. Provide it, backed by
    the ctypes hook from trn_agent_boot (if present)."""
    if "antenv.axon_hooks" in sys.modules:
        return
    try:
        import antenv
    except ImportError:
        return
    mod = types.ModuleType("antenv.axon_hooks")
    mod._hook = None
    mod.set_axon_ntff_profile_hook = lambda h: setattr(mod, "_hook", h)
    mod.get_axon_ntff_profile_hook = lambda: mod._hook
    sys.modules["antenv.axon_hooks"] = mod
    antenv.axon_hooks = mod
    try:
        from trn_agent_boot.trn_boot import _ntff_profile_via_ctypes

        h = _ntff_profile_via_ctypes("/opt/axon/libaxon_pjrt.so")
        if h is not None:
            mod._hook = h
    except Exception:
        pass


def _build_program(has_bqk: bool, has_bv: bool):
    import concourse.tile as tile
    from concourse import bacc, mybir

    bf = mybir.dt.bfloat16
    f32 = mybir.dt.float32
    EXP = mybir.ActivationFunctionType.Exp

    nc = bacc.Bacc("TRN2", target_bir_lowering=False, debug=False,
                   num_devices=NCORES)

    d = types.SimpleNamespace()
    d.xT = nc.dram_tensor("xT", [D, S], bf, kind="ExternalInput").ap()
    d.wqk = nc.dram_tensor("wqk", [D, 512], bf, kind="ExternalInput").ap()
    d.wv = nc.dram_tensor("wv", [D, 260], bf, kind="ExternalInput").ap()
    d.bqk = nc.dram_tensor("bqk", [4, 128], bf, kind="ExternalInput").ap()
    d.bv = nc.dram_tensor("bv", [1, 260], bf, kind="ExternalInput").ap()
    d.erb = nc.dram_tensor("erb", [HPC, S, S], bf, kind="ExternalInput").ap()
    d.wo = nc.dram_tensor("wo", [2, 128, D], bf, kind="ExternalInput").ap()
    d.out = nc.dram_tensor("out", [S, D], f32, kind="ExternalOutput").ap()

    st = types.SimpleNamespace()
    with tile.TileContext(nc) as tc:
        with ExitStack() as ctx:
            ep = ctx.enter_context
            # --- pools -------------------------------------------------
            xt_pool = ep(tc.tile_pool(name="xt", bufs=KC))
            wqk_pool = ep(tc.tile_pool(name="wqk", bufs=KC))
            wv_pool = ep(tc.tile_pool(name="wv", bufs=KC))
            wo_pool = ep(tc.tile_pool(name="wo", bufs=2))
            const_pool = ep(tc.tile_pool(name="consts", bufs=1))
            qkT_pool = ep(tc.tile_pool(name="qkT", bufs=4))
            v_pool = ep(tc.tile_pool(name="vsb", bufs=NSC))
            attnT_pool = ep(tc.tile_pool(name="attnT", bufs=2))
            esc_pool = ep(tc.tile_pool(name="esc", bufs=6))
            erb_pool = ep(tc.tile_pool(name="erb", bufs=10))
            pr_pool = ep(tc.tile_pool(name="prob", bufs=6))
            den_pool = ep(tc.tile_pool(name="den", bufs=3))
            bc_pool = ep(tc.tile_pool(name="bc", bufs=2))
            osb_pool = ep(tc.tile_pool(name="osb", bufs=4))
            fill_ps = ep(tc.tile_pool(name="fill_ps", bufs=1, space="PSUM"))
            sc_ps = ep(tc.tile_pool(name="sc_ps", bufs=3, space="PSUM"))
            pv_ps = ep(tc.tile_pool(name="pv_ps", bufs=4, space="PSUM"))

            # --- consts ------------------------------------------------
            st.ones_row = const_pool.tile([1, 512], bf)
            nc.gpsimd.memset(st.ones_row[:], 1.0)

            # --- loads (emission order = rough completion order) -------
            st.wqk_t, st.xt_t, st.wv_t = [], [], []
            for k in range(KC):
                w = wqk_pool.tile([128, 512], bf)
                nc.sync.dma_start(w[:], d.wqk[k * 128:(k + 1) * 128, :])
                st.wqk_t.append(w)
                xt = xt_pool.tile([128, S], bf)
                nc.sync.dma_start(xt[:], d.xT[k * 128:(k + 1) * 128, :])
                st.xt_t.append(xt)
            for k in range(KC):
                wv = wv_pool.tile([128, 260], bf)
                nc.sync.dma_start(wv[:], d.wv[k * 128:(k + 1) * 128, :])
                st.wv_t.append(wv)
            st.wo_t = []
            for p in range(2):
                w = wo_pool.tile([128, D], bf)
                nc.sync.dma_start(w[:], d.wo[p])
                st.wo_t.append(w)
            if has_bqk:
                st.bqk_sb = []
                for m in range(4):
                    t = const_pool.tile([1, 128], bf, name=f"bqk{m}",
                                        tag=f"bqk{m}")
                    nc.sync.dma_start(t[:], d.bqk[m:m + 1, :])
                    st.bqk_sb.append(t)
            if has_bv:
                st.bv_sb = const_pool.tile([1, 260], bf)
                nc.sync.dma_start(st.bv_sb[:], d.bv[:])

            # --- persistent result tiles -------------------------------
            st.qkT_t = [qkT_pool.tile([128, S], bf, name="qkT", tag="qkT")
                        for _ in range(4)]
            st.v_t = [v_pool.tile([128, 260], bf, name="vsb", tag="vsb")
                      for _ in range(NSC)]
            st.attnT_t = [attnT_pool.tile([128, S], bf, name="attnT",
                                          tag="attnT") for _ in range(2)]

            # --- filler units ------------------------------------------
            def qk_unit(m, s4):
                """One qk-projection chain: qkT[m][:, s4*512:...]."""
                ps = fill_ps.tile([128, 512], f32, name="fps", tag="fps")
                for k in range(KC):
                    nc.tensor.matmul(
                        ps[:],
                        st.wqk_t[k][:, m * 128:(m + 1) * 128],
                        st.xt_t[k][:, s4 * 512:(s4 + 1) * 512],
                        start=(k == 0),
                        stop=(k == KC - 1 and not has_bqk),
                    )
                if has_bqk:
                    nc.tensor.matmul(
                        ps[:], st.bqk_sb[m][:], st.ones_row[:, :],
                        start=False, stop=True,
                    )
                nc.vector.tensor_copy(
                    st.qkT_t[m][:, s4 * 512:(s4 + 1) * 512], ps[:])

            def v_unit(si):
                """One v-projection chain: v_t[si] (4x 64 v-cols + ones)."""
                ps = fill_ps.tile([128, 512], f32, name="fps", tag="fps")
                for k in range(KC):
                    nc.tensor.matmul(
                        ps[:, 0:260],
                        st.xt_t[k][:, si * 128:(si + 1) * 128],
                        st.wv_t[k][:],
                        start=(k == 0),
                        stop=(k == KC - 1 and not has_bv),
                    )
                if has_bv:
                    nc.tensor.matmul(
                        ps[:, 0:260], st.ones_row[0:1, 0:128], st.bv_sb[:],
                        start=False, stop=True,
                    )
                nc.vector.tensor_copy(st.v_t[si][:], ps[:, 0:260])
                for h in range(HPC):
                    nc.gpsimd.memset(
                        st.v_t[si][:, 65 * h + 64:65 * h + 65], 1.0)

            def out_unit(si, e2, pool=None):
                """One out-projection tile: out[si*128:.., e2*512:..]."""
                if pool is None:
                    ps = fill_ps.tile([128, 512], f32, name="fps", tag="fps")
                else:
                    ps = pool.tile([128, 512], f32, name="sc", tag="sc")
                for p in range(2):
                    nc.tensor.matmul(
                        ps[:],
                        st.attnT_t[p][:, si * 128:(si + 1) * 128],
                        st.wo_t[p][:, e2 * 512:(e2 + 1) * 512],
                        start=(p == 0), stop=(p == 1),
                    )
                osb = osb_pool.tile([128, 512], f32, name="osb", tag="osb")
                if e2 == 0:
                    nc.vector.tensor_copy(osb[:], ps[:])
                else:
                    nc.scalar.copy(osb[:], ps[:])
                nc.sync.dma_start(
                    d.out[si * 128:(si + 1) * 128,
                          e2 * 512:(e2 + 1) * 512],
                    osb[:])

            # --- one attention window (p = head pair, qi8 = q half) ----
            def window(p, qi8, fills):
                """fills: list of (min_unit_idx, emit_fn), FIFO order."""
                w0 = qi8 * 1024
                w1 = w0 + 1024
                nkj = w1 // 128
                units = [(kj, off) for kj in range(nkj)
                         for off in range(max(w0, (kj * 128) // 512 * 512),
                                          w1, 512)]
                n = len(units)
                pv = [[pv_ps.tile([65, 512], f32, name="pv", tag="pv")
                       for _ in range(2)] for _ in range(2)]
                erb_t = {}
                pr_t = {}

                def emit_erb(i):
                    kj, off = units[i]
                    ts = []
                    for h in range(2):
                        hl = 2 * p + h
                        rb = erb_pool.tile([128, 512], bf, name="erb",
                                           tag="erb")
                        nc.sync.dma_start(
                            rb[:],
                            d.erb[hl, kj * 128:(kj + 1) * 128, off:off + 512])
                        ts.append(rb)
                    erb_t[i] = ts

                def emit_sem(i):
                    kj, off = units[i]
                    qT = st.qkT_t[2 * p]
                    kT = st.qkT_t[2 * p + 1]
                    sc = [sc_ps.tile([128, 512], f32, name="sc", tag="sc")
                          for _ in range(2)]
                    for h in range(2):
                        rows = slice(64 * h, 64 * h + 64)
                        nc.tensor.matmul(
                            sc[h][:],
                            kT[rows, kj * 128:(kj + 1) * 128],
                            qT[rows, off:off + 512],
                            start=True, stop=True,
                            tile_position=(64 * h, 0),
                        )
                    prs = []
                    for h in range(2):
                        esc = esc_pool.tile([128, 512], bf, name="esc",
                                            tag="esc")
                        nc.scalar.activation(esc[:], sc[h][:], EXP)
                        pr = pr_pool.tile([128, 512], bf, name="prob",
                                          tag="prob")
                        nc.vector.tensor_mul(pr[:], esc[:], erb_t[i][h][:])
                        prs.append(pr)
                    del erb_t[i]
                    pr_t[i] = prs

                def emit_norm(q4):
                    den = []
                    for h in range(2):
                        t = den_pool.tile([1, 512], f32, name=f"den{h}",
                                          tag=f"den{h}")
                        nc.vector.tensor_copy(t[:], pv[h][q4][64:65, :])
                        den.append(t)
                    for h in range(2):
                        bcd = bc_pool.tile([64, 512], f32, name=f"bcd{h}",
                                           tag=f"bcd{h}")
                        nc.gpsimd.partition_broadcast(bcd[:], den[h][:])
                        bcr = bc_pool.tile([64, 512], f32, name=f"bc{h}",
                                           tag=f"bc{h}")
                        nc.vector.reciprocal_approx_fast(bcr[:], bcd[:])
                        nc.vector.tensor_mul(
                            st.attnT_t[p][64 * h:64 * h + 64,
                                          w0 + q4 * 512:w0 + (q4 + 1) * 512],
                            pv[h][q4][0:64, :],
                            bcr[:])

                def emit_pv(i):
                    kj, off = units[i]
                    q4 = (off - w0) // 512
                    last_kj = 8 * qi8 + 4 * q4 + 3
                    for h in range(2):
                        hl = 2 * p + h
                        nc.tensor.matmul(
                            pv[h][q4][:],
                            st.v_t[kj][:, 65 * hl:65 * hl + 65],
                            pr_t[i][h][:],
                            start=(kj == 0),
                            stop=(kj == last_kj),
                        )
                    del pr_t[i]
                    if kj == last_kj:
                        emit_norm(q4)

                PF = 3  # erb prefetch distance in units
                for i in range(min(PF, n)):
                    emit_erb(i)
                emit_sem(0)
                nf = len(fills)
                fi = 0
                for i in range(n):
                    if i + PF < n:
                        emit_erb(i + PF)
                    if i + 1 < n:
                        emit_sem(i + 1)
                    target = (nf * (i + 1)) // n
                    while (fi < nf and fi < target
                           and fills[fi][0] <= i):
                        fills[fi][1]()
                        fi += 1
                    emit_pv(i)
                while fi < nf:
                    fills[fi][1]()
                    fi += 1

            # --- the schedule ------------------------------------------
            def F(fn, *a):
                return (0, (lambda: fn(*a)))

            # pre-phase: the minimum needed for window (p0, q0) kj0.
            qk_unit(0, 0)
            qk_unit(0, 1)
            qk_unit(1, 0)
            v_unit(0)

            window(0, 0, [
                F(v_unit, 1), F(v_unit, 2), F(qk_unit, 1, 1),
                F(v_unit, 3), F(qk_unit, 2, 0), F(v_unit, 4),
                F(qk_unit, 2, 1), F(v_unit, 5), F(qk_unit, 3, 0),
                F(v_unit, 6), F(v_unit, 7),
            ])
            window(1, 0, [
                F(qk_unit, 3, 1), F(qk_unit, 0, 2), F(v_unit, 8),
                F(qk_unit, 0, 3), F(v_unit, 9), F(v_unit, 10),
                F(v_unit, 11), F(v_unit, 12), F(v_unit, 13),
            ])
            w2_fills = [
                F(qk_unit, 1, 2), F(qk_unit, 2, 2), F(qk_unit, 1, 3),
                F(qk_unit, 2, 3), F(qk_unit, 3, 2), F(qk_unit, 3, 3),
                F(v_unit, 14), F(v_unit, 15),
            ]
            for si in range(0, 4):
                for e2 in range(2):
                    w2_fills.append((2, (lambda si=si, e2=e2:
                                         out_unit(si, e2))))
            window(0, 1, w2_fills)
            w3_fills = []
            for si in range(4, 8):
                for e2 in range(2):
                    w3_fills.append((0, (lambda si=si, e2=e2:
                                         out_unit(si, e2))))
            window(1, 1, w3_fills)
            # tail: alternate between the filler bank and the (now idle)
            # score banks so the units pipeline instead of serializing.
            for i, (si, e2) in enumerate(
                    [(si, e2) for si in range(8, NSC) for e2 in range(2)]):
                out_unit(si, e2, pool=None if i % 2 == 0 else sc_ps)

    nc.compile()
    return nc


_PROGRAM_CACHE = {}


def _get_program(has_bqk, has_bv):
    key = (has_bqk, has_bv)
    if key not in _PROGRAM_CACHE:
        _PROGRAM_CACHE[key] = _build_program(has_bqk, has_bv)
    return _PROGRAM_CACHE[key]


_last_results = None  # BassKernelResults of the most recent run (for test.py)


def kernel(x, rel_bias, w_qkv, b_qkv, w_out, b_out, *, trace=False):
    global _last_results
    _install_ntff_hook()
    from concourse.bass_utils import run_bass_kernel_spmd

    x = np.asarray(x, dtype=np.float32)
    rel_bias = np.asarray(rel_bias, dtype=np.float32)
    w_qkv = np.asarray(w_qkv, dtype=np.float32)
    b_qkv = np.asarray(b_qkv, dtype=np.float32)
    w_out = np.asarray(w_out, dtype=np.float32)
    b_out = np.asarray(b_out, dtype=np.float32)

    wq = w_qkv[:, 0:D]
    wk = w_qkv[:, D:2 * D]
    wv = w_qkv[:, 2 * D:3 * D]
    bq, bk, bv = b_qkv[0:D], b_qkv[D:2 * D], b_qkv[2 * D:3 * D]
    has_bqk = bool(np.any(bq)) or bool(np.any(bk))
    has_bv = bool(np.any(bv))

    nc = _get_program(has_bqk, has_bv)

    sc = 1.0 / math.sqrt(HD)  # folded into the q projection
    xT = [np.ascontiguousarray(x[b].T).astype(_BF16) for b in range(B)]
    tri = np.triu(np.ones((S, S), dtype=np.float32))  # [kj, qi]: qi >= kj

    in_maps = []
    for c in range(NCORES):
        b, hg = divmod(c, 4)
        hs = [4 * hg + i for i in range(HPC)]

        # wqk columns: [q_h0 | q_h1 | k_h0 | k_h1 | q_h2 | q_h3 | k_h2 | k_h3]
        cols = []
        bqk_rows = []
        for pair in range(2):
            h0, h1 = hs[2 * pair], hs[2 * pair + 1]
            cols += [wq[:, HD * h0:HD * (h0 + 1)] * sc,
                     wq[:, HD * h1:HD * (h1 + 1)] * sc]
            bqk_rows.append(np.concatenate(
                [bq[HD * h0:HD * (h0 + 1)], bq[HD * h1:HD * (h1 + 1)]]) * sc)
            cols += [wk[:, HD * h0:HD * (h0 + 1)],
                     wk[:, HD * h1:HD * (h1 + 1)]]
            bqk_rows.append(np.concatenate(
                [bk[HD * h0:HD * (h0 + 1)], bk[HD * h1:HD * (h1 + 1)]]))
        wqk_c = np.concatenate(cols, axis=1).astype(_BF16)
        bqk_c = np.stack(bqk_rows).astype(_BF16)

        wv_c = np.zeros((D, 260), dtype=np.float32)
        bv_c = np.zeros((1, 260), dtype=np.float32)
        for i, h in enumerate(hs):
            wv_c[:, 65 * i:65 * i + 64] = wv[:, HD * h:HD * (h + 1)]
            bv_c[0, 65 * i:65 * i + 64] = bv[HD * h:HD * (h + 1)]

        erb_c = np.empty((HPC, S, S), dtype=_BF16)
        for i, h in enumerate(hs):
            erb_c[i] = (np.exp(rel_bias[h].T) * tri).astype(_BF16)

        in_maps.append({
            "xT": xT[b],
            "wqk": wqk_c,
            "wv": wv_c.astype(_BF16),
            "bqk": bqk_c,
            "bv": bv_c.astype(_BF16),
            "erb": erb_c,
            "wo": np.ascontiguousarray(
                w_out[256 * hg:256 * (hg + 1)].reshape(2, 128, D)).astype(_BF16),
        })

    res = run_bass_kernel_spmd(nc, in_maps, list(range(NCORES)), trace=trace)
    _last_results = res

    out = np.zeros((B, S, D), dtype=np.float32)
    for c in range(NCORES):
        out[c // 4] += res.results[c]["out"]
    out += b_out
    return out
